# revision 6
# baseline (speedup 1.0000x reference)
"""Trainium2 Bass kernel for nn_EnhancedPINN (pairwise-MLP PINN).

kernel(**inputs) takes the FULL unsharded inputs of reference.setup_inputs()
and returns the FULL output (forces [640,3] f32, energy scalar f32), computed
on 8 NeuronCores.

Strategy: the pairwise MLP's input is the scalar r per pair, so the network
is a 1-D function of r.  Each core (1) evaluates the exact MLP at 128
Chebyshev r-nodes (fp32 TensorE matmuls), (2) fits degree-47 Chebyshev
polynomials for g(s)=f_mag/r and u(s)=u_pair in s=r^2 via a matmul against a
host-precomputed data-independent least-squares operator, (3) evaluates the
polynomials per pair with Clenshaw on VectorE.  The static i<j triangular
pair list turns gathers into diagonal strided DMAs and segment sums into
free-dim reductions over [128 atoms x W neighbours] rectangles, once grouped
by i and once mirrored by j.  Rectangle columns are sharded across the 8
cores; the host sums per-core partials (the all-reduce).
"""

import numpy as np

N_ATOMS = 640
N_CORES = 8
EPS = 1e-3
DEG = 47
NNODES = 128
S_LO, S_HI = 0.25, 9.0
FAR = 1.0e6
SH_I, SH_J = 208, 96
WI = [80, 64, 48, 32, 16]
WJ = [16, 32, 48, 64, 80]

_cache = {}


def _split_waits(nc, limit=1):
    """Walrus in this toolchain only encodes one sync-wait per instruction;
    split excess waits onto preceding EventSemaphore instructions."""
    import concourse.mybir as mybir
    ctr = 0
    for f in nc.m.functions:
        for b in f.blocks:
            out, changed = [], False
            for inst in b.instructions:
                si = inst.sync_info
                if si is not None and si.on_wait is not None and len(si.on_wait) > limit:
                    waits = list(si.on_wait)
                    for i in range(limit, len(waits), limit):
                        ctr += 1
                        ev = mybir.InstEventSemaphore(
                            name=f"I-splitw{ctr}", ins=[], outs=[],
                            sync_info=mybir.SyncInfo(
                                on_wait=waits[i:i + limit], on_update=[]))
                        ev.engine = inst.engine
                        out.append(ev)
                    si.on_wait = waits[:limit]
                    changed = True
                out.append(inst)
            if changed:
                b.instructions = out


def _build_host_data(positions, params, i_idx, j_idx, L, rc):
    pos = np.asarray(positions, np.float32)
    assert pos.shape == (N_ATOMS, 3)
    assert int(L) == 13 and int(rc) == 3
    iu, ju = np.triu_indices(N_ATOMS, k=1)
    assert np.array_equal(np.asarray(i_idx), iu.astype(np.int32)), \
        "kernel specialized for static i<j triu pair list"
    assert np.array_equal(np.asarray(j_idx), ju.astype(np.int32))

    def W(d):
        return np.asarray(d['w'], np.float32)

    def B(d):
        return np.asarray(d['b'], np.float32)

    def fold(lnp, w, b):
        g = np.asarray(lnp['g'], np.float32)
        bb = np.asarray(lnp['bb'], np.float32)
        return (g[:, None] * w).astype(np.float32), (b + bb @ w).astype(np.float32)

    enc, fh, eh = params['enc'], params['force'], params['energy']
    w_e1, b_e1 = W(enc['d1']), B(enc['d1'])
    w_e2, b_e2 = fold(enc['ln1'], W(enc['d2']), B(enc['d2']))

    def head(hp):
        w1, b1 = fold(enc['ln2'], W(hp['d1']), B(hp['d1']))
        w2, b2 = fold(hp['ln1'], W(hp['d2']), B(hp['d2']))
        w3, b3 = fold(hp['ln2'], W(hp['d3']), B(hp['d3']))
        w4, b4 = fold(hp['ln3'], W(hp['d4']), B(hp['d4']))
        return [(w1, b1), (w2, b2), (w3, b3), (w4, b4)]

    layers_f, layers_e = head(fh), head(eh)

    # Chebyshev nodes + fit operator (data independent)
    k = np.arange(NNODES)
    tk = np.cos(np.pi * (k + 0.5) / NNODES)
    mid, half = 0.5 * (S_LO + S_HI), 0.5 * (S_HI - S_LO)
    s_nodes = mid + half * tk
    r_nodes = np.sqrt(s_nodes)
    V = np.polynomial.chebyshev.chebvander(tk, DEG)
    Pinv = np.linalg.pinv(V)
    Pinv_g = Pinv / r_nodes[None, :]
    lhsT_fit = np.concatenate([Pinv_g.T, Pinv.T], axis=1).astype(np.float32)

    pos_pad = np.full((1408, 3), FAR, np.float32)
    pos_pad[:N_ATOMS] = pos

    pos_sh_i = np.zeros((N_CORES, 5, 3, SH_I), np.float32)
    pos_sh_j = np.zeros((N_CORES, 5, 3, SH_J), np.float32)
    thresh_j = np.zeros((N_CORES, 128, 5), np.float32)
    pvec = np.arange(128)
    for c in range(N_CORES):
        for g in range(5):
            b0 = c * WI[g] + 128 * g + 1
            pos_sh_i[c, g] = pos_pad[b0:b0 + SH_I].T
            pos_sh_j[c, g] = pos_pad[c * WJ[g]:c * WJ[g] + SH_J].T
            thresh_j[c, :, g] = (128 * g + pvec) - c * WJ[g]
    pi_cols = np.zeros((128, 15), np.float32)
    for g in range(5):
        for comp in range(3):
            pi_cols[:, 3 * g + comp] = pos[128 * g + pvec, comp]

    # mask thresholds matched bit-exactly to the reference's sqrt-based masks
    f32 = np.float32
    dr0 = (pos[ju] - pos[iu]).astype(f32)
    m1 = (dr0 > f32(6.5)).astype(f32)
    m2 = (dr0 < f32(-6.5)).astype(f32)
    dr = (m1 * f32(-13.0) + dr0).astype(f32)
    dr = (m2 * f32(13.0) + dr).astype(f32)
    s = (dr[:, 0] * dr[:, 0]).astype(f32)
    s = (s + (dr[:, 1] * dr[:, 1]).astype(f32)).astype(f32)
    s = (s + (dr[:, 2] * dr[:, 2]).astype(f32)).astype(f32)
    r32 = np.sqrt(s, dtype=f32)
    lt3 = r32 < f32(3.0)
    gt05 = r32 > f32(0.5)

    def pick(amax, bmin, default):
        if np.isfinite(amax) and np.isfinite(bmin):
            assert amax < bmin
            return 0.5 * (float(amax) + float(bmin))
        return default

    t_hi = pick(s[lt3].max() if np.any(lt3) else -np.inf,
                s[~lt3].min() if np.any(~lt3) else np.inf, 9.0)
    t_lo = pick(s[~gt05].max() if np.any(~gt05) else -np.inf,
                s[gt05].min() if np.any(gt05) else np.inf, 0.25)

    return dict(
        w_e1=w_e1, b_e1=b_e1, w_e2=w_e2, b_e2=b_e2,
        layers_f=layers_f, layers_e=layers_e,
        r_nodes=r_nodes.astype(np.float32), lhsT_fit=lhsT_fit,
        pos=pos, pos_sh_i=pos_sh_i, pos_sh_j=pos_sh_j, thresh_j=thresh_j,
        pi_cols=pi_cols,
        t_lo=float(t_lo), t_hi=float(t_hi),
    )


def _trace_kernel(hd):
    import concourse.bass as bass
    import concourse.mybir as mybir
    import concourse.tile as tile
    from concourse.ap import AP

    F32 = mybir.dt.float32
    Alu = mybir.AluOpType
    Act = mybir.ActivationFunctionType

    nc = bass.Bass("TRN2")
    DP1 = DEG + 1
    cri = np.concatenate([[0], np.cumsum(WI)]).astype(int)
    crj = np.concatenate([[0], np.cumsum(WJ)]).astype(int)
    WTOT = int(cri[-1])

    def dram_in(name, shape):
        return nc.dram_tensor(name, list(shape), F32, kind="ExternalInput")

    pi_cols_d = dram_in("pi_cols", [128, 15])
    posshi = dram_in("pos_sh_i", [5 * SH_I * 3])
    posshj = dram_in("pos_sh_j", [5 * SH_J * 3])
    threshj = dram_in("thresh_j", [128, 5])
    iota_d = dram_in("iota", [80])
    ident_d = dram_in("ident", [128, 128])
    ones_d = dram_in("ones_row", [1, 128])
    rnodes_d = dram_in("rnodes_row", [1, 128])
    fit_d = dram_in("lhsT_fit", [128, 2 * DP1])
    we1_d = dram_in("w_e1", [1, 256])
    be1_d = dram_in("b_e1", [1, 256])
    we2_d = dram_in("w_e2", [256, 256])
    be2_d = dram_in("b_e2", [1, 256])
    wf_d, bf_d, weh_d, beh_d = [], [], [], []
    for li in range(4):
        w, b = hd['layers_f'][li]
        wf_d.append(dram_in(f"w_f{li}", list(w.shape)))
        bf_d.append(dram_in(f"b_f{li}", [1, w.shape[1]]))
        w, b = hd['layers_e'][li]
        weh_d.append(dram_in(f"w_eh{li}", list(w.shape)))
        beh_d.append(dram_in(f"b_eh{li}", [1, w.shape[1]]))

    forces_i_o = nc.dram_tensor("forces_i", [128, 15], F32, kind="ExternalOutput")
    forces_j_o = nc.dram_tensor("forces_j", [128, 15], F32, kind="ExternalOutput")
    energy_o = nc.dram_tensor("energy", [1, 1], F32, kind="ExternalOutput")

    T_LO, T_HI = hd['t_lo'], hd['t_hi']
    MID, HALF = 0.5 * (S_LO + S_HI), 0.5 * (S_HI - S_LO)

    with tile.TileContext(nc) as tc:
        with (
            tc.tile_pool(name="wpool", bufs=1) as wpool,
            tc.tile_pool(name="node", bufs=1) as npool,
            tc.tile_pool(name="pair", bufs=1) as ppool,
            tc.tile_pool(name="ps", bufs=1, space="PSUM") as pspool,
            tc.tile_pool(name="ps2", bufs=2, space="PSUM") as pspool2,
        ):
            def load(shape, src_ap, name):
                t = wpool.tile(list(shape), F32, tag=name)
                nc.sync.dma_start(t[:], src_ap)
                return t

            def load_w(dram, din_, dout, name):
                t = wpool.tile([128, din_ // 128, dout], F32, tag=name)
                nc.sync.dma_start(t[:], dram[:].rearrange("(a p) c -> p a c", p=128))
                return t

            ident = load([128, 128], ident_d[:], "ident")
            ones_row = load([1, 128], ones_d[:], "ones")
            rnodes_row = load([1, 128], rnodes_d[:], "rnodes")
            fit_sb = load([128, 2 * DP1], fit_d[:], "fit")
            we1 = load([1, 256], we1_d[:], "we1")
            be1 = load([1, 256], be1_d[:], "be1")
            we2 = load_w(we2_d, 256, 256, "we2")
            be2 = load([1, 256], be2_d[:], "be2")
            wfs, bfs, wes, bes = [], [], [], []
            for li in range(4):
                w, b = hd['layers_f'][li]
                wfs.append(load_w(wf_d[li], w.shape[0], w.shape[1], f"wf{li}"))
                bfs.append(load([1, w.shape[1]], bf_d[li][:], f"bf{li}"))
                w, b = hd['layers_e'][li]
                wes.append(load_w(weh_d[li], w.shape[0], w.shape[1], f"wes{li}"))
                bes.append(load([1, w.shape[1]], beh_d[li][:], f"bes{li}"))

            thresh_sb = load([128, 5], threshj[:], "thresh")
            iota_sb = ppool.tile([128, 80], F32, tag="iota")
            nc.sync.dma_start(iota_sb[:],
                               iota_d[:].unsqueeze(0).partition_broadcast(128))

            # ---------- node MLP (exact, fp32) ----------
            x_cur = npool.tile([128, 512], F32, tag="x_cur")
            y_sb = npool.tile([128, 512], F32, tag="y_sb")
            xT = npool.tile([128, 512], F32, tag="xT")
            feats_T = npool.tile([128, 256], F32, tag="featsT")
            sqs = npool.tile([128, 512], F32, tag="sqs")
            stat = npool.tile([128, 8], F32, tag="stat")
            fit_rhs = npool.tile([128, 2], F32, tag="fit_rhs")
            nc.vector.memset(stat[:, 7:8], 1.5)

            y_ps = pspool.tile([128, 512], F32, tag="y_ps")
            t_ps = pspool2.tile([128, 128], F32, tag="t_ps")

            def layer_norm(d):
                nc.vector.scalar_tensor_tensor(
                    sqs[:, :d], y_sb[:, :d], 1.0, y_sb[:, :d],
                    Alu.mult, Alu.mult, accum_out=stat[:, 1:2])
                nc.vector.tensor_scalar_mul(stat[:, 2:3], stat[:, 0:1], 1.0 / d)
                nc.vector.tensor_tensor(stat[:, 5:6], stat[:, 2:3], stat[:, 2:3],
                                        Alu.mult)
                nc.vector.scalar_tensor_tensor(
                    stat[:, 3:4], stat[:, 1:2], 1.0 / d, stat[:, 5:6],
                    Alu.mult, Alu.subtract)
                nc.vector.tensor_scalar_add(stat[:, 3:4], stat[:, 3:4], EPS)
                nc.scalar.activation(stat[:, 5:6], stat[:, 3:4], Act.Sqrt)
                nc.vector.reciprocal(stat[:, 4:5], stat[:, 5:6])
                # one Newton polish of inv = 1/sqrt(var+eps)
                nc.vector.tensor_tensor(stat[:, 5:6], stat[:, 4:5], stat[:, 4:5],
                                        Alu.mult)
                nc.vector.tensor_tensor(stat[:, 5:6], stat[:, 5:6], stat[:, 3:4],
                                        Alu.mult)
                nc.vector.scalar_tensor_tensor(
                    stat[:, 5:6], stat[:, 5:6], -0.5, stat[:, 7:8],
                    Alu.mult, Alu.add)
                nc.vector.tensor_tensor(stat[:, 4:5], stat[:, 4:5], stat[:, 5:6],
                                        Alu.mult)
                nc.vector.tensor_scalar(
                    x_cur[:, :d], y_sb[:, :d], stat[:, 2:3], stat[:, 4:5],
                    Alu.subtract, Alu.mult)

            def transpose_to(xt_tile, src_tile, d):
                for kc in range(d // 128):
                    sl = slice(kc * 128, kc * 128 + 128)
                    nc.tensor.transpose(t_ps[:], src_tile[:, sl], ident[:])
                    nc.scalar.copy(xt_tile[:, sl], t_ps[:])

            def dense(xt_tile, w_t, b_t, din_, dout, out_col=None, tanh=True):
                nc.tensor.matmul(y_ps[:, :dout], ones_row[:], b_t[:],
                                 start=True, stop=False)
                nk = din_ // 128
                for kc in range(nk):
                    nc.tensor.matmul(y_ps[:, :dout],
                                     xt_tile[:, kc * 128:(kc + 1) * 128],
                                     w_t[:, kc, :],
                                     start=False, stop=(kc == nk - 1))
                if tanh:
                    nc.scalar.activation(y_sb[:, :dout], y_ps[:, :dout], Act.Tanh,
                                         accum_out=stat[:, 0:1])
                else:
                    nc.scalar.copy(out_col, y_ps[:, :dout])

            nc.tensor.matmul(y_ps[:, :256], ones_row[:], be1[:],
                             start=True, stop=False)
            nc.tensor.matmul(y_ps[:, :256], rnodes_row[:], we1[:],
                             start=False, stop=True)
            nc.scalar.activation(y_sb[:, :256], y_ps[:, :256], Act.Tanh,
                                 accum_out=stat[:, 0:1])
            layer_norm(256)
            transpose_to(xT, x_cur, 256)
            dense(xT, we2, be2, 256, 256)
            layer_norm(256)
            transpose_to(feats_T, x_cur, 256)
            for (ws, bs, outc) in ((wfs, bfs, 0), (wes, bes, 1)):
                d3 = hd['layers_f' if outc == 0 else 'layers_e'][2][0].shape[1]
                dims = [(256, 512), (512, 512), (512, d3), (d3, 1)]
                xt = feats_T
                for li in range(3):
                    di, do = dims[li]
                    dense(xt, ws[li], bs[li], di, do)
                    layer_norm(do)
                    transpose_to(xT, x_cur, do)
                    xt = xT
                dense(xt, ws[3], bs[3], dims[3][0], 1,
                      out_col=fit_rhs[:, outc:outc + 1], tanh=False)

            # ---------- Chebyshev fit ----------
            fit_ps = pspool.tile([2 * DP1, 2], F32, tag="fit_ps")
            nc.tensor.matmul(fit_ps[:], fit_sb[:], fit_rhs[:],
                             start=True, stop=True)
            fit_c = npool.tile([2 * DP1, 2], F32, tag="fit_c")
            nc.scalar.copy(fit_c[:], fit_ps[:])
            crow_ps = pspool.tile([1, 2 * DP1], F32, tag="crow_ps")
            crow_g = npool.tile([1, 2 * DP1], F32, tag="crow_g")
            crow_u = npool.tile([1, 2 * DP1], F32, tag="crow_u")
            nc.tensor.matmul(crow_ps[:], fit_c[:, 0:1],
                             ident[0:2 * DP1, 0:2 * DP1], is_transpose=True)
            nc.scalar.copy(crow_g[:], crow_ps[:])
            nc.tensor.matmul(crow_ps[:], fit_c[:, 1:2],
                             ident[0:2 * DP1, 0:2 * DP1], is_transpose=True)
            nc.scalar.copy(crow_u[:], crow_ps[:])
            cb_ps = pspool.tile([128, 2 * DP1], F32, tag="cb_ps")
            nc.tensor.matmul(cb_ps[:, 0:DP1], ones_row[:], crow_g[0:1, 0:DP1],
                             start=True, stop=True)
            nc.tensor.matmul(cb_ps[:, DP1:2 * DP1], ones_row[:],
                             crow_u[0:1, DP1:2 * DP1], start=True, stop=True)
            coefT = npool.tile([128, 2 * DP1], F32, tag="coefT")
            nc.scalar.copy(coefT[:], cb_ps[:])

            def cg(kk):
                return coefT[:, kk:kk + 1]

            def cu(kk):
                return coefT[:, DP1 + kk:DP1 + kk + 1]

            # ---------- per-pair phase ----------
            pj = [ppool.tile([128, WTOT], F32, tag=f"pj{c}", name=f"pj{c}") for c in range(3)]
            mA = ppool.tile([128, WTOT], F32, tag="mA")
            mB = ppool.tile([128, WTOT], F32, tag="mB")
            s_t = ppool.tile([128, WTOT], F32, tag="s_t")
            tmp = ppool.tile([128, WTOT], F32, tag="tmp")
            mk = ppool.tile([128, WTOT], F32, tag="mk")
            t_t = ppool.tile([128, WTOT], F32, tag="t_t")
            t2 = ppool.tile([128, WTOT], F32, tag="t2")
            cb = [ppool.tile([128, WTOT], F32, tag=f"cb{c}", name=f"cbt{c}") for c in range(3)]
            pg = ppool.tile([128, WTOT], F32, tag="pg")
            pu = ppool.tile([128, WTOT], F32, tag="pu")
            scr = ppool.tile([128, WTOT], F32, tag="scr")
            piS = ppool.tile([128, 16], F32, tag="piS")
            nc.sync.dma_start(piS[:, 0:15], pi_cols_d[:])
            fsb = [ppool.tile([128, 15], F32, tag=f"fsb{c}", name=f"fsb{c}") for c in range(2)]
            ecol = ppool.tile([128, 1], F32, tag="ecol")
            ones_col = ppool.tile([128, 1], F32, tag="ones_col")
            nc.vector.memset(ones_col[:], 1.0)
            e_ps = pspool.tile([1, 1], F32, tag="e_ps")
            e_sb = ppool.tile([1, 1], F32, tag="e_sb")

            def clenshaw(out_t, coef, deg):
                b1, b2, b3 = cb
                nc.vector.tensor_scalar(b1[:], t2[:], 0.0, coef(deg),
                                        Alu.mult, Alu.add)
                nc.vector.memset(b2[:], 0.0)
                for kk in range(deg - 1, 0, -1):
                    nc.vector.tensor_tensor(tmp[:], b1[:], t2[:], Alu.mult)
                    nc.vector.scalar_tensor_tensor(
                        b3[:], tmp[:], coef(kk), b2[:], Alu.add, Alu.subtract)
                    b1, b2, b3 = b3, b1, b2
                nc.vector.tensor_tensor(tmp[:], b1[:], t_t[:], Alu.mult)
                nc.vector.scalar_tensor_tensor(
                    out_t[:], tmp[:], coef(0), b2[:], Alu.add, Alu.subtract)

            for side in range(2):
                Wg = WI if side == 0 else WJ
                cr = cri if side == 0 else crj
                sh_d = posshi if side == 0 else posshj
                SH = SH_I if side == 0 else SH_J
                for g in range(5):
                    sl = slice(int(cr[g]), int(cr[g + 1]))
                    for c in range(3):
                        base = (g * 3 + c) * SH
                        if side == 0:
                            ap = AP(sh_d, base, [[1, 128], [1, Wg[g]]])
                            nc.sync.dma_start(pj[c][:, sl], ap)
                        else:
                            ap = AP(sh_d, base, [[1, Wg[g]]])
                            nc.sync.dma_start(
                                pj[c][:, sl],
                                ap.unsqueeze(0).partition_broadcast(128))
                for g in range(5):
                    sl = slice(int(cr[g]), int(cr[g + 1]))
                    for c in range(3):
                        nc.vector.tensor_scalar(
                            pj[c][:, sl], pj[c][:, sl],
                            piS[:, 3 * g + c:3 * g + c + 1], None, Alu.subtract)
                for c in range(3):
                    nc.vector.tensor_scalar(mA[:], pj[c][:], 6.5, None, Alu.is_gt)
                    nc.vector.tensor_scalar(mB[:], pj[c][:], -6.5, None, Alu.is_lt)
                    nc.vector.scalar_tensor_tensor(
                        pj[c][:], mA[:], -13.0, pj[c][:], Alu.mult, Alu.add)
                    nc.vector.scalar_tensor_tensor(
                        pj[c][:], mB[:], 13.0, pj[c][:], Alu.mult, Alu.add)
                nc.vector.tensor_tensor(s_t[:], pj[0][:], pj[0][:], Alu.mult)
                nc.vector.tensor_tensor(tmp[:], pj[1][:], pj[1][:], Alu.mult)
                nc.vector.tensor_tensor(s_t[:], s_t[:], tmp[:], Alu.add)
                nc.vector.tensor_tensor(tmp[:], pj[2][:], pj[2][:], Alu.mult)
                nc.vector.tensor_tensor(s_t[:], s_t[:], tmp[:], Alu.add)
                nc.vector.tensor_scalar(mk[:], s_t[:], T_HI, None, Alu.is_lt)
                nc.vector.scalar_tensor_tensor(
                    mk[:], s_t[:], T_LO, mk[:], Alu.is_gt, Alu.mult)
                if side == 1:
                    for g in range(5):
                        sl = slice(int(cr[g]), int(cr[g + 1]))
                        nc.vector.scalar_tensor_tensor(
                            mk[:, sl], iota_sb[:, 0:Wg[g]],
                            thresh_sb[:, g:g + 1], mk[:, sl],
                            Alu.is_lt, Alu.mult)
                nc.vector.tensor_scalar(s_t[:], s_t[:], S_LO, S_HI,
                                        Alu.max, Alu.min)
                nc.vector.tensor_scalar(t_t[:], s_t[:], MID, 1.0 / HALF,
                                        Alu.subtract, Alu.mult)
                nc.vector.tensor_scalar_mul(t2[:], t_t[:], 2.0)
                clenshaw(pg, cg, DEG)
                if side == 0:
                    clenshaw(pu, cu, DEG)
                    nc.vector.scalar_tensor_tensor(
                        scr[:], pu[:], 1.0, mk[:], Alu.mult, Alu.mult,
                        accum_out=ecol[:, 0:1])
                    nc.tensor.matmul(e_ps[:], ecol[:], ones_col[:],
                                     start=True, stop=True)
                    nc.scalar.copy(e_sb[:], e_ps[:])
                    nc.sync.dma_start(energy_o[:], e_sb[:])
                nc.vector.tensor_tensor(pg[:], pg[:], mk[:], Alu.mult)
                for g in range(5):
                    sl = slice(int(cr[g]), int(cr[g + 1]))
                    for c in range(3):
                        nc.vector.scalar_tensor_tensor(
                            scr[:, sl], pg[:, sl], 1.0, pj[c][:, sl],
                            Alu.mult, Alu.mult,
                            accum_out=fsb[side][:, 3 * g + c:3 * g + c + 1])
                nc.sync.dma_start((forces_i_o if side == 0 else forces_j_o)[:],
                                  fsb[side][:])

    _split_waits(nc)
    return nc


def kernel(positions, params, i_idx, j_idx, L, rc):
    hd = _build_host_data(positions, params, i_idx, j_idx, L, rc)

    key = ("k", hd['t_lo'], hd['t_hi'])
    if key not in _cache:
        _cache[key] = _trace_kernel(hd)
    nc = _cache[key]

    base = {
        "pi_cols": hd['pi_cols'],
        "iota": np.arange(80, dtype=np.float32),
        "ident": np.eye(128, dtype=np.float32),
        "ones_row": np.ones((1, 128), np.float32),
        "rnodes_row": hd['r_nodes'].reshape(1, 128),
        "lhsT_fit": hd['lhsT_fit'],
        "w_e1": hd['w_e1'].reshape(1, 256), "b_e1": hd['b_e1'].reshape(1, 256),
        "w_e2": hd['w_e2'], "b_e2": hd['b_e2'].reshape(1, 256),
    }
    for li in range(4):
        w, b = hd['layers_f'][li]
        base[f"w_f{li}"] = w
        base[f"b_f{li}"] = b.reshape(1, -1)
        w, b = hd['layers_e'][li]
        base[f"w_eh{li}"] = w
        base[f"b_eh{li}"] = b.reshape(1, -1)

    in_maps = []
    for c in range(N_CORES):
        m = dict(base)
        m["pos_sh_i"] = hd['pos_sh_i'][c].reshape(-1)
        m["pos_sh_j"] = hd['pos_sh_j'][c].reshape(-1)
        m["thresh_j"] = hd['thresh_j'][c]
        in_maps.append(m)

    from concourse.bass_utils import run_bass_kernel_spmd
    res = run_bass_kernel_spmd(nc, in_maps, core_ids=list(range(N_CORES)))
    global LAST_EXEC_NS
    LAST_EXEC_NS = res.exec_time_ns

    forces = np.zeros((N_ATOMS, 3), np.float64)
    energy = 0.0
    for c in range(N_CORES):
        r = res.results[c]
        fi = r["forces_i"].astype(np.float64)
        fj = r["forces_j"].astype(np.float64)
        for g in range(5):
            for comp in range(3):
                forces[128 * g:128 * (g + 1), comp] += fi[:, 3 * g + comp]
                forces[128 * g:128 * (g + 1), comp] += fj[:, 3 * g + comp]
        energy += float(r["energy"][0, 0])
    return forces.astype(np.float32), np.float32(energy)


# revision 8
# speedup vs baseline: 1.6296x; 1.6296x over previous
"""Trainium2 Bass kernel for nn_EnhancedPINN (pairwise-MLP PINN).

kernel(**inputs) takes the FULL unsharded inputs of reference.setup_inputs()
and returns the FULL output (forces [640,3] f32, energy scalar f32), computed
on 8 NeuronCores.

Strategy: the pairwise MLP's input is the scalar r per pair, so the network
is a 1-D function of r.  Each core (1) evaluates the exact MLP at 128
Chebyshev r-nodes (fp32 TensorE matmuls), (2) fits degree-47 Chebyshev
polynomials for g(s)=f_mag/r and u(s)=u_pair in s=r^2 via a matmul against a
host-precomputed data-independent least-squares operator, (3) evaluates the
polynomials per pair with Clenshaw on VectorE.  The static i<j triangular
pair list turns gathers into diagonal strided DMAs and segment sums into
free-dim reductions over [128 atoms x W neighbours] rectangles, once grouped
by i and once mirrored by j.  Rectangle columns are sharded across the 8
cores; the host sums per-core partials (the all-reduce).
"""

import numpy as np

N_ATOMS = 640
N_CORES = 8
EPS = 1e-3
DEG = 31
NNODES = 128
S_LO, S_HI = 0.25, 9.0
FAR = 1.0e6
SH_I, SH_J = 208, 96
WI = [80, 64, 48, 32, 16]
WJ = [16, 32, 48, 64, 80]

_cache = {}


def _split_waits(nc, limit=1):
    """Walrus in this toolchain only encodes one sync-wait per instruction;
    split excess waits onto preceding EventSemaphore instructions."""
    import concourse.mybir as mybir
    ctr = 0
    for f in nc.m.functions:
        for b in f.blocks:
            out, changed = [], False
            for inst in b.instructions:
                si = inst.sync_info
                if si is not None and si.on_wait is not None and len(si.on_wait) > limit:
                    waits = list(si.on_wait)
                    for i in range(limit, len(waits), limit):
                        ctr += 1
                        ev = mybir.InstEventSemaphore(
                            name=f"I-splitw{ctr}", ins=[], outs=[],
                            sync_info=mybir.SyncInfo(
                                on_wait=waits[i:i + limit], on_update=[]))
                        ev.engine = inst.engine
                        out.append(ev)
                    si.on_wait = waits[:limit]
                    changed = True
                out.append(inst)
            if changed:
                b.instructions = out


def _build_host_data(positions, params, i_idx, j_idx, L, rc):
    pos = np.asarray(positions, np.float32)
    assert pos.shape == (N_ATOMS, 3)
    assert int(L) == 13 and int(rc) == 3
    iu, ju = np.triu_indices(N_ATOMS, k=1)
    assert np.array_equal(np.asarray(i_idx), iu.astype(np.int32)), \
        "kernel specialized for static i<j triu pair list"
    assert np.array_equal(np.asarray(j_idx), ju.astype(np.int32))

    def W(d):
        return np.asarray(d['w'], np.float32)

    def B(d):
        return np.asarray(d['b'], np.float32)

    def fold(lnp, w, b):
        g = np.asarray(lnp['g'], np.float32)
        bb = np.asarray(lnp['bb'], np.float32)
        return (g[:, None] * w).astype(np.float32), (b + bb @ w).astype(np.float32)

    enc, fh, eh = params['enc'], params['force'], params['energy']
    w_e1, b_e1 = W(enc['d1']), B(enc['d1'])
    w_e2, b_e2 = fold(enc['ln1'], W(enc['d2']), B(enc['d2']))

    def head(hp):
        w1, b1 = fold(enc['ln2'], W(hp['d1']), B(hp['d1']))
        w2, b2 = fold(hp['ln1'], W(hp['d2']), B(hp['d2']))
        w3, b3 = fold(hp['ln2'], W(hp['d3']), B(hp['d3']))
        w4, b4 = fold(hp['ln3'], W(hp['d4']), B(hp['d4']))
        return [(w1, b1), (w2, b2), (w3, b3), (w4, b4)]

    layers_f, layers_e = head(fh), head(eh)

    # Chebyshev nodes + fit operator (data independent)
    k = np.arange(NNODES)
    tk = np.cos(np.pi * (k + 0.5) / NNODES)
    mid, half = 0.5 * (S_LO + S_HI), 0.5 * (S_HI - S_LO)
    s_nodes = mid + half * tk
    r_nodes = np.sqrt(s_nodes)
    V = np.polynomial.chebyshev.chebvander(tk, DEG)
    Pinv = np.linalg.pinv(V)
    Pinv_g = Pinv / r_nodes[None, :]
    lhsT_fit = np.concatenate([Pinv_g.T, Pinv.T], axis=1).astype(np.float32)

    pos_pad = np.full((1408, 3), FAR, np.float32)
    pos_pad[:N_ATOMS] = pos

    pos_sh_i = np.zeros((N_CORES, 5, 3, SH_I), np.float32)
    pos_sh_j = np.zeros((N_CORES, 5, 3, SH_J), np.float32)
    thresh_j = np.zeros((N_CORES, 128, 5), np.float32)
    pvec = np.arange(128)
    for c in range(N_CORES):
        for g in range(5):
            b0 = c * WI[g] + 128 * g + 1
            pos_sh_i[c, g] = pos_pad[b0:b0 + SH_I].T
            pos_sh_j[c, g] = pos_pad[c * WJ[g]:c * WJ[g] + SH_J].T
            thresh_j[c, :, g] = (128 * g + pvec) - c * WJ[g]
    pi_cols = np.zeros((128, 15), np.float32)
    for g in range(5):
        for comp in range(3):
            pi_cols[:, 3 * g + comp] = pos[128 * g + pvec, comp]

    # mask thresholds matched bit-exactly to the reference's sqrt-based masks
    f32 = np.float32
    dr0 = (pos[ju] - pos[iu]).astype(f32)
    m1 = (dr0 > f32(6.5)).astype(f32)
    m2 = (dr0 < f32(-6.5)).astype(f32)
    dr = (m1 * f32(-13.0) + dr0).astype(f32)
    dr = (m2 * f32(13.0) + dr).astype(f32)
    s = (dr[:, 0] * dr[:, 0]).astype(f32)
    s = (s + (dr[:, 1] * dr[:, 1]).astype(f32)).astype(f32)
    s = (s + (dr[:, 2] * dr[:, 2]).astype(f32)).astype(f32)
    r32 = np.sqrt(s, dtype=f32)
    lt3 = r32 < f32(3.0)
    gt05 = r32 > f32(0.5)

    def pick(amax, bmin, default):
        if np.isfinite(amax) and np.isfinite(bmin):
            assert amax < bmin
            return 0.5 * (float(amax) + float(bmin))
        return default

    t_hi = pick(s[lt3].max() if np.any(lt3) else -np.inf,
                s[~lt3].min() if np.any(~lt3) else np.inf, 9.0)
    t_lo = pick(s[~gt05].max() if np.any(~gt05) else -np.inf,
                s[gt05].min() if np.any(gt05) else np.inf, 0.25)

    return dict(
        w_e1=w_e1, b_e1=b_e1, w_e2=w_e2, b_e2=b_e2,
        layers_f=layers_f, layers_e=layers_e,
        r_nodes=r_nodes.astype(np.float32), lhsT_fit=lhsT_fit,
        pos=pos, pos_sh_i=pos_sh_i, pos_sh_j=pos_sh_j, thresh_j=thresh_j,
        pi_cols=pi_cols,
        t_lo=float(t_lo), t_hi=float(t_hi),
    )


def _trace_kernel(hd):
    import concourse.bass as bass
    import concourse.mybir as mybir
    import concourse.tile as tile
    from concourse.ap import AP

    F32 = mybir.dt.float32
    Alu = mybir.AluOpType
    Act = mybir.ActivationFunctionType

    nc = bass.Bass("TRN2")
    DP1 = DEG + 1
    cri = np.concatenate([[0], np.cumsum(WI)]).astype(int)
    crj = np.concatenate([[0], np.cumsum(WJ)]).astype(int)
    WTOT = int(cri[-1])

    def dram_in(name, shape):
        return nc.dram_tensor(name, list(shape), F32, kind="ExternalInput")

    pi_cols_d = dram_in("pi_cols", [128, 15])
    posshi = dram_in("pos_sh_i", [5 * SH_I * 3])
    posshj = dram_in("pos_sh_j", [5 * SH_J * 3])
    threshj = dram_in("thresh_j", [128, 5])
    iota_d = dram_in("iota", [80])
    ident_d = dram_in("ident", [128, 128])
    ones_d = dram_in("ones_row", [1, 128])
    rnodes_d = dram_in("rnodes_row", [1, 128])
    fit_d = dram_in("lhsT_fit", [128, 2 * DP1])
    we1_d = dram_in("w_e1", [1, 256])
    be1_d = dram_in("b_e1", [1, 256])
    we2_d = dram_in("w_e2", [256, 256])
    be2_d = dram_in("b_e2", [1, 256])
    wf_d, bf_d, weh_d, beh_d = [], [], [], []
    for li in range(4):
        w, b = hd['layers_f'][li]
        wf_d.append(dram_in(f"w_f{li}", list(w.shape)))
        bf_d.append(dram_in(f"b_f{li}", [1, w.shape[1]]))
        w, b = hd['layers_e'][li]
        weh_d.append(dram_in(f"w_eh{li}", list(w.shape)))
        beh_d.append(dram_in(f"b_eh{li}", [1, w.shape[1]]))

    forces_i_o = nc.dram_tensor("forces_i", [128, 15], F32, kind="ExternalOutput")
    forces_j_o = nc.dram_tensor("forces_j", [128, 15], F32, kind="ExternalOutput")
    energy_o = nc.dram_tensor("energy", [1, 1], F32, kind="ExternalOutput")

    T_LO, T_HI = hd['t_lo'], hd['t_hi']
    MID, HALF = 0.5 * (S_LO + S_HI), 0.5 * (S_HI - S_LO)

    with tile.TileContext(nc) as tc:
        with (
            tc.tile_pool(name="wpool", bufs=1) as wpool,
            tc.tile_pool(name="node", bufs=1) as npool,
            tc.tile_pool(name="pair", bufs=1) as ppool,
            tc.tile_pool(name="ps", bufs=1, space="PSUM") as pspool,
            tc.tile_pool(name="ps2", bufs=2, space="PSUM") as pspool2,
        ):
            def load(shape, src_ap, name):
                t = wpool.tile(list(shape), F32, tag=name)
                nc.sync.dma_start(t[:], src_ap)
                return t

            def load_w(dram, din_, dout, name):
                t = wpool.tile([128, din_ // 128, dout], F32, tag=name)
                nc.sync.dma_start(t[:], dram[:].rearrange("(a p) c -> p a c", p=128))
                return t

            ident = load([128, 128], ident_d[:], "ident")
            ones_row = load([1, 128], ones_d[:], "ones")
            rnodes_row = load([1, 128], rnodes_d[:], "rnodes")
            fit_sb = load([128, 2 * DP1], fit_d[:], "fit")
            we1 = load([1, 256], we1_d[:], "we1")
            be1 = load([1, 256], be1_d[:], "be1")
            we2 = load_w(we2_d, 256, 256, "we2")
            be2 = load([1, 256], be2_d[:], "be2")
            wfs, bfs, wes, bes = [], [], [], []
            for li in range(4):
                w, b = hd['layers_f'][li]
                wfs.append(load_w(wf_d[li], w.shape[0], w.shape[1], f"wf{li}"))
                bfs.append(load([1, w.shape[1]], bf_d[li][:], f"bf{li}"))
                w, b = hd['layers_e'][li]
                wes.append(load_w(weh_d[li], w.shape[0], w.shape[1], f"wes{li}"))
                bes.append(load([1, w.shape[1]], beh_d[li][:], f"bes{li}"))

            thresh_sb = load([128, 5], threshj[:], "thresh")
            iota_sb = ppool.tile([128, 80], F32, tag="iota")
            nc.sync.dma_start(iota_sb[:],
                               iota_d[:].unsqueeze(0).partition_broadcast(128))

            # ---------- node MLP (exact, fp32), per-head tile contexts ----------
            fit_rhs = npool.tile([128, 2], F32, tag="fit_rhs")

            def mkctx(h):
                ctx = dict(
                    x_cur=npool.tile([128, 512], F32, tag=f"x_cur{h}", name=f"x_cur{h}"),
                    y_sb=npool.tile([128, 512], F32, tag=f"y_sb{h}", name=f"y_sb{h}"),
                    xT=npool.tile([128, 512], F32, tag=f"xT{h}", name=f"xT{h}"),
                    sqs=npool.tile([128, 512], F32, tag=f"sqs{h}", name=f"sqs{h}"),
                    stat=npool.tile([128, 8], F32, tag=f"stat{h}", name=f"stat{h}"),
                    y_ps=pspool.tile([128, 512], F32, tag=f"y_ps{h}", name=f"y_ps{h}"),
                    t_ps=pspool.tile([128, 128], F32, tag=f"t_ps{h}", name=f"t_ps{h}"),
                )
                nc.vector.memset(ctx['stat'][:, 7:8], 1.5)
                return ctx

            ctx_f = mkctx("f")
            ctx_e = mkctx("e")

            def layer_norm(cx, d):
                y_sb, sqs, stat, x_cur = cx['y_sb'], cx['sqs'], cx['stat'], cx['x_cur']
                nc.vector.scalar_tensor_tensor(
                    sqs[:, :d], y_sb[:, :d], 1.0, y_sb[:, :d],
                    Alu.mult, Alu.mult, accum_out=stat[:, 1:2])
                nc.vector.tensor_scalar_mul(stat[:, 2:3], stat[:, 0:1], 1.0 / d)
                nc.vector.tensor_tensor(stat[:, 5:6], stat[:, 2:3], stat[:, 2:3],
                                        Alu.mult)
                nc.vector.scalar_tensor_tensor(
                    stat[:, 3:4], stat[:, 1:2], 1.0 / d, stat[:, 5:6],
                    Alu.mult, Alu.subtract)
                nc.vector.tensor_scalar_add(stat[:, 3:4], stat[:, 3:4], EPS)
                nc.scalar.activation(stat[:, 5:6], stat[:, 3:4], Act.Sqrt)
                nc.vector.reciprocal(stat[:, 4:5], stat[:, 5:6])
                nc.vector.tensor_tensor(stat[:, 5:6], stat[:, 4:5], stat[:, 4:5],
                                        Alu.mult)
                nc.vector.tensor_tensor(stat[:, 5:6], stat[:, 5:6], stat[:, 3:4],
                                        Alu.mult)
                nc.vector.scalar_tensor_tensor(
                    stat[:, 5:6], stat[:, 5:6], -0.5, stat[:, 7:8],
                    Alu.mult, Alu.add)
                nc.vector.tensor_tensor(stat[:, 4:5], stat[:, 4:5], stat[:, 5:6],
                                        Alu.mult)
                nc.vector.tensor_scalar(
                    x_cur[:, :d], y_sb[:, :d], stat[:, 2:3], stat[:, 4:5],
                    Alu.subtract, Alu.mult)

            def transpose_to(cx, xt_tile, src_tile, d):
                for kc in range(d // 128):
                    sl = slice(kc * 128, kc * 128 + 128)
                    nc.tensor.transpose(cx['t_ps'][:], src_tile[:, sl], ident[:])
                    nc.scalar.copy(xt_tile[:, sl], cx['t_ps'][:])

            def dense(cx, xt_tile, w_t, b_t, din_, dout, out_col=None, tanh=True):
                y_ps, y_sb, stat = cx['y_ps'], cx['y_sb'], cx['stat']
                nc.tensor.matmul(y_ps[:, :dout], ones_row[:], b_t[:],
                                 start=True, stop=False)
                nk = din_ // 128
                for kc in range(nk):
                    nc.tensor.matmul(y_ps[:, :dout],
                                     xt_tile[:, kc * 128:(kc + 1) * 128],
                                     w_t[:, kc, :],
                                     start=False, stop=(kc == nk - 1))
                if tanh:
                    nc.scalar.activation(y_sb[:, :dout], y_ps[:, :dout], Act.Tanh,
                                         accum_out=stat[:, 0:1])
                else:
                    nc.scalar.copy(out_col, y_ps[:, :dout])

            # shared encoder (on the force-head context)
            feats_T = npool.tile([128, 256], F32, tag="featsT")
            cxf = ctx_f
            nc.tensor.matmul(cxf['y_ps'][:, :256], ones_row[:], be1[:],
                             start=True, stop=False)
            nc.tensor.matmul(cxf['y_ps'][:, :256], rnodes_row[:], we1[:],
                             start=False, stop=True)
            nc.scalar.activation(cxf['y_sb'][:, :256], cxf['y_ps'][:, :256],
                                 Act.Tanh, accum_out=cxf['stat'][:, 0:1])
            layer_norm(cxf, 256)
            transpose_to(cxf, cxf['xT'], cxf['x_cur'], 256)
            dense(cxf, cxf['xT'], we2, be2, 256, 256)
            layer_norm(cxf, 256)
            transpose_to(cxf, feats_T, cxf['x_cur'], 256)
            # heads: emit layer-by-layer interleaved, independent tile sets
            plans = []
            for (cx, ws, bs, outc) in ((ctx_f, wfs, bfs, 0), (ctx_e, wes, bes, 1)):
                d3 = hd['layers_f' if outc == 0 else 'layers_e'][2][0].shape[1]
                dims = [(256, 512), (512, 512), (512, d3), (d3, 1)]
                plans.append((cx, ws, bs, outc, dims))
            for li in range(3):
                for (cx, ws, bs, outc, dims) in plans:
                    di, do = dims[li]
                    xt = feats_T if li == 0 else cx['xT']
                    dense(cx, xt, ws[li], bs[li], di, do)
                    layer_norm(cx, do)
                    transpose_to(cx, cx['xT'], cx['x_cur'], do)
            for (cx, ws, bs, outc, dims) in plans:
                dense(cx, cx['xT'], ws[3], bs[3], dims[3][0], 1,
                      out_col=fit_rhs[:, outc:outc + 1], tanh=False)

            # ---------- Chebyshev fit ----------
            fit_ps = pspool.tile([2 * DP1, 2], F32, tag="fit_ps")
            nc.tensor.matmul(fit_ps[:], fit_sb[:], fit_rhs[:],
                             start=True, stop=True)
            fit_c = npool.tile([2 * DP1, 2], F32, tag="fit_c")
            nc.scalar.copy(fit_c[:], fit_ps[:])
            crow_ps = pspool.tile([1, 2 * DP1], F32, tag="crow_ps")
            crow_g = npool.tile([1, 2 * DP1], F32, tag="crow_g")
            crow_u = npool.tile([1, 2 * DP1], F32, tag="crow_u")
            nc.tensor.matmul(crow_ps[:], fit_c[:, 0:1],
                             ident[0:2 * DP1, 0:2 * DP1], is_transpose=True)
            nc.scalar.copy(crow_g[:], crow_ps[:])
            nc.tensor.matmul(crow_ps[:], fit_c[:, 1:2],
                             ident[0:2 * DP1, 0:2 * DP1], is_transpose=True)
            nc.scalar.copy(crow_u[:], crow_ps[:])
            cb_ps = pspool.tile([128, 2 * DP1], F32, tag="cb_ps")
            nc.tensor.matmul(cb_ps[:, 0:DP1], ones_row[:], crow_g[0:1, 0:DP1],
                             start=True, stop=True)
            nc.tensor.matmul(cb_ps[:, DP1:2 * DP1], ones_row[:],
                             crow_u[0:1, DP1:2 * DP1], start=True, stop=True)
            coefT = npool.tile([128, 2 * DP1], F32, tag="coefT")
            nc.scalar.copy(coefT[:], cb_ps[:])

            def cg(kk):
                return coefT[:, kk:kk + 1]

            def cu(kk):
                return coefT[:, DP1 + kk:DP1 + kk + 1]

            # ---------- per-pair phase (i-side cols 0:240, j-side 240:480) ----------
            WT2 = 2 * WTOT
            pj = [ppool.tile([128, WT2], F32, tag=f"pj{c}", name=f"pj{c}") for c in range(3)]
            mA = ppool.tile([128, WT2], F32, tag="mA")
            mB = ppool.tile([128, WT2], F32, tag="mB")
            s_t = ppool.tile([128, WT2], F32, tag="s_t")
            tmp = ppool.tile([128, WT2], F32, tag="tmp")
            mk = ppool.tile([128, WT2], F32, tag="mk")
            t_t = ppool.tile([128, WT2], F32, tag="t_t")
            t2 = ppool.tile([128, WT2], F32, tag="t2")
            cb = [ppool.tile([128, WT2], F32, tag=f"cb{c}", name=f"cbt{c}") for c in range(3)]
            pg = ppool.tile([128, WT2], F32, tag="pg")
            pu = ppool.tile([128, WTOT], F32, tag="pu")
            scr = ppool.tile([128, WT2], F32, tag="scr")
            piS = ppool.tile([128, 16], F32, tag="piS")
            nc.sync.dma_start(piS[:, 0:15], pi_cols_d[:])
            fsb = ppool.tile([128, 30], F32, tag="fsb")
            ecol = ppool.tile([128, 1], F32, tag="ecol")
            ones_col = ppool.tile([128, 1], F32, tag="ones_col")
            nc.vector.memset(ones_col[:], 1.0)
            e_ps = pspool.tile([1, 1], F32, tag="e_ps")
            e_sb = ppool.tile([1, 1], F32, tag="e_sb")

            # group column ranges across the merged tile
            GRP = []   # (colslice, g, is_j)
            for g in range(5):
                GRP.append((slice(int(cri[g]), int(cri[g + 1])), g, False, WI[g]))
            for g in range(5):
                GRP.append((slice(WTOT + int(crj[g]), WTOT + int(crj[g + 1])),
                            g, True, WJ[g]))

            for (sl, g, isj, Wg) in GRP:
                SH = SH_J if isj else SH_I
                sh_d = posshj if isj else posshi
                for c in range(3):
                    base = (g * 3 + c) * SH
                    if isj:
                        ap = AP(sh_d, base, [[1, Wg]])
                        nc.sync.dma_start(pj[c][:, sl],
                                          ap.unsqueeze(0).partition_broadcast(128))
                    else:
                        ap = AP(sh_d, base, [[1, 128], [1, Wg]])
                        nc.sync.dma_start(pj[c][:, sl], ap)
            for (sl, g, isj, Wg) in GRP:
                for c in range(3):
                    nc.vector.tensor_scalar(
                        pj[c][:, sl], pj[c][:, sl],
                        piS[:, 3 * g + c:3 * g + c + 1], None, Alu.subtract)
            for c in range(3):
                nc.vector.tensor_scalar(mA[:], pj[c][:], 6.5, None, Alu.is_gt)
                nc.vector.tensor_scalar(mB[:], pj[c][:], -6.5, None, Alu.is_lt)
                nc.vector.scalar_tensor_tensor(
                    pj[c][:], mA[:], -13.0, pj[c][:], Alu.mult, Alu.add)
                nc.vector.scalar_tensor_tensor(
                    pj[c][:], mB[:], 13.0, pj[c][:], Alu.mult, Alu.add)
            nc.vector.tensor_tensor(s_t[:], pj[0][:], pj[0][:], Alu.mult)
            nc.vector.tensor_tensor(tmp[:], pj[1][:], pj[1][:], Alu.mult)
            nc.vector.tensor_tensor(s_t[:], s_t[:], tmp[:], Alu.add)
            nc.vector.tensor_tensor(tmp[:], pj[2][:], pj[2][:], Alu.mult)
            nc.vector.tensor_tensor(s_t[:], s_t[:], tmp[:], Alu.add)
            nc.vector.tensor_scalar(mk[:], s_t[:], T_HI, None, Alu.is_lt)
            nc.vector.scalar_tensor_tensor(
                mk[:], s_t[:], T_LO, mk[:], Alu.is_gt, Alu.mult)
            for (sl, g, isj, Wg) in GRP:
                if isj:
                    nc.vector.scalar_tensor_tensor(
                        mk[:, sl], iota_sb[:, 0:Wg],
                        thresh_sb[:, g:g + 1], mk[:, sl], Alu.is_lt, Alu.mult)
            nc.vector.tensor_scalar(s_t[:], s_t[:], S_LO, S_HI, Alu.max, Alu.min)
            nc.vector.tensor_scalar(t_t[:], s_t[:], MID, 1.0 / HALF,
                                    Alu.subtract, Alu.mult)
            nc.vector.tensor_scalar_mul(t2[:], t_t[:], 2.0)

            def clenshaw(out_t, coef, deg, width):
                b1, b2, b3 = cb
                nc.vector.tensor_scalar(b1[:, :width], t2[:, :width], 0.0,
                                        coef(deg), Alu.mult, Alu.add)
                nc.vector.memset(b2[:, :width], 0.0)
                for kk in range(deg - 1, 0, -1):
                    nc.vector.tensor_tensor(tmp[:, :width], b1[:, :width],
                                            t2[:, :width], Alu.mult)
                    nc.vector.scalar_tensor_tensor(
                        b3[:, :width], tmp[:, :width], coef(kk), b2[:, :width],
                        Alu.add, Alu.subtract)
                    b1, b2, b3 = b3, b1, b2
                nc.vector.tensor_tensor(tmp[:, :width], b1[:, :width],
                                        t_t[:, :width], Alu.mult)
                nc.vector.scalar_tensor_tensor(
                    out_t[:, :width], tmp[:, :width], coef(0), b2[:, :width],
                    Alu.add, Alu.subtract)

            clenshaw(pg, cg, DEG, WT2)
            clenshaw(pu, cu, DEG, WTOT)
            nc.vector.scalar_tensor_tensor(
                scr[:, :WTOT], pu[:], 1.0, mk[:, :WTOT], Alu.mult, Alu.mult,
                accum_out=ecol[:, 0:1])
            nc.tensor.matmul(e_ps[:], ecol[:], ones_col[:], start=True, stop=True)
            nc.scalar.copy(e_sb[:], e_ps[:])
            nc.sync.dma_start(energy_o[:], e_sb[:])
            nc.vector.tensor_tensor(pg[:], pg[:], mk[:], Alu.mult)
            for gi, (sl, g, isj, Wg) in enumerate(GRP):
                for c in range(3):
                    col = (15 if isj else 0) + 3 * g + c
                    nc.vector.scalar_tensor_tensor(
                        scr[:, sl], pg[:, sl], 1.0, pj[c][:, sl],
                        Alu.mult, Alu.mult,
                        accum_out=fsb[:, col:col + 1])
            nc.sync.dma_start(forces_i_o[:], fsb[:, 0:15])
            nc.sync.dma_start(forces_j_o[:], fsb[:, 15:30])

    _split_waits(nc)
    return nc


def kernel(positions, params, i_idx, j_idx, L, rc):
    hd = _build_host_data(positions, params, i_idx, j_idx, L, rc)

    key = ("k", hd['t_lo'], hd['t_hi'])
    if key not in _cache:
        _cache[key] = _trace_kernel(hd)
    nc = _cache[key]

    base = {
        "pi_cols": hd['pi_cols'],
        "iota": np.arange(80, dtype=np.float32),
        "ident": np.eye(128, dtype=np.float32),
        "ones_row": np.ones((1, 128), np.float32),
        "rnodes_row": hd['r_nodes'].reshape(1, 128),
        "lhsT_fit": hd['lhsT_fit'],
        "w_e1": hd['w_e1'].reshape(1, 256), "b_e1": hd['b_e1'].reshape(1, 256),
        "w_e2": hd['w_e2'], "b_e2": hd['b_e2'].reshape(1, 256),
    }
    for li in range(4):
        w, b = hd['layers_f'][li]
        base[f"w_f{li}"] = w
        base[f"b_f{li}"] = b.reshape(1, -1)
        w, b = hd['layers_e'][li]
        base[f"w_eh{li}"] = w
        base[f"b_eh{li}"] = b.reshape(1, -1)

    in_maps = []
    for c in range(N_CORES):
        m = dict(base)
        m["pos_sh_i"] = hd['pos_sh_i'][c].reshape(-1)
        m["pos_sh_j"] = hd['pos_sh_j'][c].reshape(-1)
        m["thresh_j"] = hd['thresh_j'][c]
        in_maps.append(m)

    from concourse.bass_utils import run_bass_kernel_spmd
    res = run_bass_kernel_spmd(nc, in_maps, core_ids=list(range(N_CORES)))
    global LAST_EXEC_NS
    LAST_EXEC_NS = res.exec_time_ns

    forces = np.zeros((N_ATOMS, 3), np.float64)
    energy = 0.0
    for c in range(N_CORES):
        r = res.results[c]
        fi = r["forces_i"].astype(np.float64)
        fj = r["forces_j"].astype(np.float64)
        for g in range(5):
            for comp in range(3):
                forces[128 * g:128 * (g + 1), comp] += fi[:, 3 * g + comp]
                forces[128 * g:128 * (g + 1), comp] += fj[:, 3 * g + comp]
        energy += float(r["energy"][0, 0])
    return forces.astype(np.float32), np.float32(energy)


# revision 10
# speedup vs baseline: 1.8463x; 1.1330x over previous
"""Trainium2 Bass kernel for nn_EnhancedPINN (pairwise-MLP PINN).

kernel(**inputs) takes the FULL unsharded inputs of reference.setup_inputs()
and returns the FULL output (forces [640,3] f32, energy scalar f32), computed
on 8 NeuronCores.

Strategy: the pairwise MLP's input is the scalar r per pair, so the network
is a 1-D function of r.  Each core (1) evaluates the exact MLP at 128
Chebyshev r-nodes (fp32 TensorE matmuls), (2) fits degree-47 Chebyshev
polynomials for g(s)=f_mag/r and u(s)=u_pair in s=r^2 via a matmul against a
host-precomputed data-independent least-squares operator, (3) evaluates the
polynomials per pair with Clenshaw on VectorE.  The static i<j triangular
pair list turns gathers into diagonal strided DMAs and segment sums into
free-dim reductions over [128 atoms x W neighbours] rectangles, once grouped
by i and once mirrored by j.  Rectangle columns are sharded across the 8
cores; the host sums per-core partials (the all-reduce).
"""

import numpy as np

N_ATOMS = 640
N_CORES = 8
EPS = 1e-3
DEG_G = 31
DEG_U = 15
NNODES = 128
S_LO, S_HI = 0.25, 9.0
FAR = 1.0e6
SH_I, SH_J = 208, 96
WI = [80, 64, 48, 32, 16]
WJ = [16, 32, 48, 64, 80]

_cache = {}


def _split_waits(nc, limit=1):
    """Walrus in this toolchain only encodes one sync-wait per instruction;
    split excess waits onto preceding EventSemaphore instructions."""
    import concourse.mybir as mybir
    ctr = 0
    for f in nc.m.functions:
        for b in f.blocks:
            out, changed = [], False
            for inst in b.instructions:
                si = inst.sync_info
                if si is not None and si.on_wait is not None and len(si.on_wait) > limit:
                    waits = list(si.on_wait)
                    for i in range(limit, len(waits), limit):
                        ctr += 1
                        ev = mybir.InstEventSemaphore(
                            name=f"I-splitw{ctr}", ins=[], outs=[],
                            sync_info=mybir.SyncInfo(
                                on_wait=waits[i:i + limit], on_update=[]))
                        ev.engine = inst.engine
                        out.append(ev)
                    si.on_wait = waits[:limit]
                    changed = True
                out.append(inst)
            if changed:
                b.instructions = out


def _build_host_data(positions, params, i_idx, j_idx, L, rc):
    pos = np.asarray(positions, np.float32)
    assert pos.shape == (N_ATOMS, 3)
    assert int(L) == 13 and int(rc) == 3
    iu, ju = np.triu_indices(N_ATOMS, k=1)
    assert np.array_equal(np.asarray(i_idx), iu.astype(np.int32)), \
        "kernel specialized for static i<j triu pair list"
    assert np.array_equal(np.asarray(j_idx), ju.astype(np.int32))

    def W(d):
        return np.asarray(d['w'], np.float32)

    def B(d):
        return np.asarray(d['b'], np.float32)

    def fold(lnp, w, b):
        g = np.asarray(lnp['g'], np.float32)
        bb = np.asarray(lnp['bb'], np.float32)
        return (g[:, None] * w).astype(np.float32), (b + bb @ w).astype(np.float32)

    enc, fh, eh = params['enc'], params['force'], params['energy']
    w_e1, b_e1 = W(enc['d1']), B(enc['d1'])
    w_e2, b_e2 = fold(enc['ln1'], W(enc['d2']), B(enc['d2']))

    def head(hp):
        w1, b1 = fold(enc['ln2'], W(hp['d1']), B(hp['d1']))
        w2, b2 = fold(hp['ln1'], W(hp['d2']), B(hp['d2']))
        w3, b3 = fold(hp['ln2'], W(hp['d3']), B(hp['d3']))
        w4, b4 = fold(hp['ln3'], W(hp['d4']), B(hp['d4']))
        return [(w1, b1), (w2, b2), (w3, b3), (w4, b4)]

    layers_f, layers_e = head(fh), head(eh)

    # Chebyshev nodes + fit operator (data independent)
    k = np.arange(NNODES)
    tk = np.cos(np.pi * (k + 0.5) / NNODES)
    mid, half = 0.5 * (S_LO + S_HI), 0.5 * (S_HI - S_LO)
    s_nodes = mid + half * tk
    r_nodes = np.sqrt(s_nodes)
    Vg = np.polynomial.chebyshev.chebvander(tk, DEG_G)
    Vu = np.polynomial.chebyshev.chebvander(tk, DEG_U)
    Pinv_g = np.linalg.pinv(Vg) / r_nodes[None, :]
    Pinv_u = np.linalg.pinv(Vu)
    lhsT_fit = np.concatenate([Pinv_g.T, Pinv_u.T], axis=1).astype(np.float32)

    pos_pad = np.full((1408, 3), FAR, np.float32)
    pos_pad[:N_ATOMS] = pos

    pos_sh_i = np.zeros((N_CORES, 5, 3, SH_I), np.float32)
    pos_sh_j = np.zeros((N_CORES, 5, 3, SH_J), np.float32)
    thresh_j = np.zeros((N_CORES, 128, 5), np.float32)
    pvec = np.arange(128)
    for c in range(N_CORES):
        for g in range(5):
            b0 = c * WI[g] + 128 * g + 1
            pos_sh_i[c, g] = pos_pad[b0:b0 + SH_I].T
            pos_sh_j[c, g] = pos_pad[c * WJ[g]:c * WJ[g] + SH_J].T
            thresh_j[c, :, g] = (128 * g + pvec) - c * WJ[g]
    pi_cols = np.zeros((128, 15), np.float32)
    for g in range(5):
        for comp in range(3):
            pi_cols[:, 3 * g + comp] = pos[128 * g + pvec, comp]

    # mask thresholds matched bit-exactly to the reference's sqrt-based masks
    f32 = np.float32
    dr0 = (pos[ju] - pos[iu]).astype(f32)
    m1 = (dr0 > f32(6.5)).astype(f32)
    m2 = (dr0 < f32(-6.5)).astype(f32)
    dr = (m1 * f32(-13.0) + dr0).astype(f32)
    dr = (m2 * f32(13.0) + dr).astype(f32)
    s = (dr[:, 0] * dr[:, 0]).astype(f32)
    s = (s + (dr[:, 1] * dr[:, 1]).astype(f32)).astype(f32)
    s = (s + (dr[:, 2] * dr[:, 2]).astype(f32)).astype(f32)
    r32 = np.sqrt(s, dtype=f32)
    lt3 = r32 < f32(3.0)
    gt05 = r32 > f32(0.5)

    def pick(amax, bmin, default):
        if np.isfinite(amax) and np.isfinite(bmin):
            assert amax < bmin
            return 0.5 * (float(amax) + float(bmin))
        return default

    t_hi = pick(s[lt3].max() if np.any(lt3) else -np.inf,
                s[~lt3].min() if np.any(~lt3) else np.inf, 9.0)
    t_lo = pick(s[~gt05].max() if np.any(~gt05) else -np.inf,
                s[gt05].min() if np.any(gt05) else np.inf, 0.25)

    return dict(
        w_e1=w_e1, b_e1=b_e1, w_e2=w_e2, b_e2=b_e2,
        layers_f=layers_f, layers_e=layers_e,
        r_nodes=r_nodes.astype(np.float32), lhsT_fit=lhsT_fit,
        pos=pos, pos_sh_i=pos_sh_i, pos_sh_j=pos_sh_j, thresh_j=thresh_j,
        pi_cols=pi_cols,
        t_lo=float(t_lo), t_hi=float(t_hi),
    )


def _trace_kernel(hd):
    import concourse.bass as bass
    import concourse.mybir as mybir
    import concourse.tile as tile
    from concourse.ap import AP

    F32 = mybir.dt.float32
    Alu = mybir.AluOpType
    Act = mybir.ActivationFunctionType

    nc = bass.Bass("TRN2")
    DP1 = DEG_G + 1
    DU1 = DEG_U + 1
    NCOEF = DP1 + DU1
    cri = np.concatenate([[0], np.cumsum(WI)]).astype(int)
    crj = np.concatenate([[0], np.cumsum(WJ)]).astype(int)
    WTOT = int(cri[-1])

    def dram_in(name, shape):
        return nc.dram_tensor(name, list(shape), F32, kind="ExternalInput")

    pi_cols_d = dram_in("pi_cols", [128, 15])
    posshi = dram_in("pos_sh_i", [5 * SH_I * 3])
    posshj = dram_in("pos_sh_j", [5 * SH_J * 3])
    threshj = dram_in("thresh_j", [128, 5])
    iota_d = dram_in("iota", [80])
    ident_d = dram_in("ident", [128, 128])
    ones_d = dram_in("ones_row", [1, 128])
    rnodes_d = dram_in("rnodes_row", [1, 128])
    fit_d = dram_in("lhsT_fit", [128, NCOEF])
    we1_d = dram_in("w_e1", [1, 256])
    be1_d = dram_in("b_e1", [1, 256])
    we2_d = dram_in("w_e2", [256, 256])
    be2_d = dram_in("b_e2", [1, 256])
    wf_d, bf_d, weh_d, beh_d = [], [], [], []
    for li in range(4):
        w, b = hd['layers_f'][li]
        wf_d.append(dram_in(f"w_f{li}", list(w.shape)))
        bf_d.append(dram_in(f"b_f{li}", [1, w.shape[1]]))
        w, b = hd['layers_e'][li]
        weh_d.append(dram_in(f"w_eh{li}", list(w.shape)))
        beh_d.append(dram_in(f"b_eh{li}", [1, w.shape[1]]))

    forces_i_o = nc.dram_tensor("forces_i", [128, 15], F32, kind="ExternalOutput")
    forces_j_o = nc.dram_tensor("forces_j", [128, 15], F32, kind="ExternalOutput")
    energy_o = nc.dram_tensor("energy", [1, 1], F32, kind="ExternalOutput")

    T_LO, T_HI = hd['t_lo'], hd['t_hi']
    MID, HALF = 0.5 * (S_LO + S_HI), 0.5 * (S_HI - S_LO)

    with tile.TileContext(nc) as tc:
        with (
            tc.tile_pool(name="wpool", bufs=1) as wpool,
            tc.tile_pool(name="node", bufs=1) as npool,
            tc.tile_pool(name="pair", bufs=1) as ppool,
            tc.tile_pool(name="ps", bufs=1, space="PSUM") as pspool,
            tc.tile_pool(name="ps2", bufs=2, space="PSUM") as pspool2,
        ):
            def load(shape, src_ap, name):
                t = wpool.tile(list(shape), F32, tag=name)
                nc.sync.dma_start(t[:], src_ap)
                return t

            def load_w(dram, din_, dout, name):
                t = wpool.tile([128, din_ // 128, dout], F32, tag=name)
                nc.sync.dma_start(t[:], dram[:].rearrange("(a p) c -> p a c", p=128))
                return t

            ident = load([128, 128], ident_d[:], "ident")
            ones_row = load([1, 128], ones_d[:], "ones")
            rnodes_row = load([1, 128], rnodes_d[:], "rnodes")
            fit_sb = load([128, NCOEF], fit_d[:], "fit")
            we1 = load([1, 256], we1_d[:], "we1")
            be1 = load([1, 256], be1_d[:], "be1")
            we2 = load_w(we2_d, 256, 256, "we2")
            be2 = load([1, 256], be2_d[:], "be2")
            wfs, bfs, wes, bes = [], [], [], []
            for li in range(4):
                w, b = hd['layers_f'][li]
                wfs.append(load_w(wf_d[li], w.shape[0], w.shape[1], f"wf{li}"))
                bfs.append(load([1, w.shape[1]], bf_d[li][:], f"bf{li}"))
                w, b = hd['layers_e'][li]
                wes.append(load_w(weh_d[li], w.shape[0], w.shape[1], f"wes{li}"))
                bes.append(load([1, w.shape[1]], beh_d[li][:], f"bes{li}"))

            thresh_sb = load([128, 5], threshj[:], "thresh")
            iota_sb = ppool.tile([128, 80], F32, tag="iota")
            nc.sync.dma_start(iota_sb[:],
                               iota_d[:].unsqueeze(0).partition_broadcast(128))

            # ---------- node MLP (exact, fp32), per-head tile contexts ----------
            fit_rhs = npool.tile([128, 2], F32, tag="fit_rhs")

            def mkctx(h):
                ctx = dict(
                    x_cur=npool.tile([128, 512], F32, tag=f"x_cur{h}", name=f"x_cur{h}"),
                    y_sb=npool.tile([128, 512], F32, tag=f"y_sb{h}", name=f"y_sb{h}"),
                    xT=npool.tile([128, 512], F32, tag=f"xT{h}", name=f"xT{h}"),
                    sqs=npool.tile([128, 512], F32, tag=f"sqs{h}", name=f"sqs{h}"),
                    stat=npool.tile([128, 8], F32, tag=f"stat{h}", name=f"stat{h}"),
                    y_ps=pspool.tile([128, 512], F32, tag=f"y_ps{h}", name=f"y_ps{h}"),
                    t_ps=pspool.tile([128, 128], F32, tag=f"t_ps{h}", name=f"t_ps{h}"),
                )
                nc.vector.memset(ctx['stat'][:, 7:8], 1.5)
                return ctx

            ctx_f = mkctx("f")
            ctx_e = mkctx("e")

            def layer_norm(cx, d):
                y_sb, sqs, stat, x_cur = cx['y_sb'], cx['sqs'], cx['stat'], cx['x_cur']
                nc.vector.scalar_tensor_tensor(
                    sqs[:, :d], y_sb[:, :d], 1.0, y_sb[:, :d],
                    Alu.mult, Alu.mult, accum_out=stat[:, 1:2])
                nc.vector.tensor_scalar_mul(stat[:, 2:3], stat[:, 0:1], 1.0 / d)
                nc.vector.tensor_tensor(stat[:, 5:6], stat[:, 2:3], stat[:, 2:3],
                                        Alu.mult)
                nc.vector.scalar_tensor_tensor(
                    stat[:, 3:4], stat[:, 1:2], 1.0 / d, stat[:, 5:6],
                    Alu.mult, Alu.subtract)
                nc.vector.tensor_scalar_add(stat[:, 3:4], stat[:, 3:4], EPS)
                nc.scalar.activation(stat[:, 5:6], stat[:, 3:4], Act.Sqrt)
                nc.vector.reciprocal(stat[:, 4:5], stat[:, 5:6])
                nc.vector.tensor_tensor(stat[:, 5:6], stat[:, 4:5], stat[:, 4:5],
                                        Alu.mult)
                nc.vector.tensor_tensor(stat[:, 5:6], stat[:, 5:6], stat[:, 3:4],
                                        Alu.mult)
                nc.vector.scalar_tensor_tensor(
                    stat[:, 5:6], stat[:, 5:6], -0.5, stat[:, 7:8],
                    Alu.mult, Alu.add)
                nc.vector.tensor_tensor(stat[:, 4:5], stat[:, 4:5], stat[:, 5:6],
                                        Alu.mult)
                nc.vector.tensor_scalar(
                    x_cur[:, :d], y_sb[:, :d], stat[:, 2:3], stat[:, 4:5],
                    Alu.subtract, Alu.mult)

            def transpose_to(cx, xt_tile, src_tile, d):
                for kc in range(d // 128):
                    sl = slice(kc * 128, kc * 128 + 128)
                    nc.tensor.transpose(cx['t_ps'][:], src_tile[:, sl], ident[:])
                    nc.scalar.copy(xt_tile[:, sl], cx['t_ps'][:])

            def dense(cx, xt_tile, w_t, b_t, din_, dout, out_col=None, tanh=True):
                y_ps, y_sb, stat = cx['y_ps'], cx['y_sb'], cx['stat']
                nc.tensor.matmul(y_ps[:, :dout], ones_row[:], b_t[:],
                                 start=True, stop=False)
                nk = din_ // 128
                for kc in range(nk):
                    nc.tensor.matmul(y_ps[:, :dout],
                                     xt_tile[:, kc * 128:(kc + 1) * 128],
                                     w_t[:, kc, :],
                                     start=False, stop=(kc == nk - 1))
                if tanh:
                    nc.scalar.activation(y_sb[:, :dout], y_ps[:, :dout], Act.Tanh,
                                         accum_out=stat[:, 0:1])
                else:
                    nc.scalar.copy(out_col, y_ps[:, :dout])

            # shared encoder (on the force-head context)
            feats_T = npool.tile([128, 256], F32, tag="featsT")
            cxf = ctx_f
            nc.tensor.matmul(cxf['y_ps'][:, :256], ones_row[:], be1[:],
                             start=True, stop=False)
            nc.tensor.matmul(cxf['y_ps'][:, :256], rnodes_row[:], we1[:],
                             start=False, stop=True)
            nc.scalar.activation(cxf['y_sb'][:, :256], cxf['y_ps'][:, :256],
                                 Act.Tanh, accum_out=cxf['stat'][:, 0:1])
            layer_norm(cxf, 256)
            transpose_to(cxf, cxf['xT'], cxf['x_cur'], 256)
            dense(cxf, cxf['xT'], we2, be2, 256, 256)
            layer_norm(cxf, 256)
            transpose_to(cxf, feats_T, cxf['x_cur'], 256)
            # heads: emit layer-by-layer interleaved, independent tile sets
            plans = []
            for (cx, ws, bs, outc) in ((ctx_f, wfs, bfs, 0), (ctx_e, wes, bes, 1)):
                d3 = hd['layers_f' if outc == 0 else 'layers_e'][2][0].shape[1]
                dims = [(256, 512), (512, 512), (512, d3), (d3, 1)]
                plans.append((cx, ws, bs, outc, dims))
            for li in range(3):
                for (cx, ws, bs, outc, dims) in plans:
                    di, do = dims[li]
                    xt = feats_T if li == 0 else cx['xT']
                    dense(cx, xt, ws[li], bs[li], di, do)
                    layer_norm(cx, do)
                    transpose_to(cx, cx['xT'], cx['x_cur'], do)
            for (cx, ws, bs, outc, dims) in plans:
                dense(cx, cx['xT'], ws[3], bs[3], dims[3][0], 1,
                      out_col=fit_rhs[:, outc:outc + 1], tanh=False)

            # ---------- Chebyshev fit ----------
            fit_ps = pspool.tile([NCOEF, 2], F32, tag="fit_ps")
            nc.tensor.matmul(fit_ps[:], fit_sb[:], fit_rhs[:],
                             start=True, stop=True)
            fit_c = npool.tile([NCOEF, 2], F32, tag="fit_c")
            nc.scalar.copy(fit_c[:], fit_ps[:])
            crow_ps = pspool.tile([1, NCOEF], F32, tag="crow_ps")
            crow_g = npool.tile([1, NCOEF], F32, tag="crow_g")
            crow_u = npool.tile([1, NCOEF], F32, tag="crow_u")
            nc.tensor.matmul(crow_ps[:], fit_c[:, 0:1],
                             ident[0:NCOEF, 0:NCOEF], is_transpose=True)
            nc.scalar.copy(crow_g[:], crow_ps[:])
            nc.tensor.matmul(crow_ps[:], fit_c[:, 1:2],
                             ident[0:NCOEF, 0:NCOEF], is_transpose=True)
            nc.scalar.copy(crow_u[:], crow_ps[:])
            cb_ps = pspool.tile([128, NCOEF], F32, tag="cb_ps")
            nc.tensor.matmul(cb_ps[:, 0:DP1], ones_row[:], crow_g[0:1, 0:DP1],
                             start=True, stop=True)
            nc.tensor.matmul(cb_ps[:, DP1:NCOEF], ones_row[:],
                             crow_u[0:1, DP1:NCOEF], start=True, stop=True)
            coefT = npool.tile([128, NCOEF], F32, tag="coefT")
            nc.scalar.copy(coefT[:], cb_ps[:])

            def cg(kk):
                return coefT[:, kk:kk + 1]

            # ---------- per-pair phase (i-side cols 0:240, j-side 240:480) ----------
            WT2 = 2 * WTOT
            pj = [ppool.tile([128, WT2], F32, tag=f"pj{c}", name=f"pj{c}") for c in range(3)]
            mA = ppool.tile([128, WT2], F32, tag="mA")
            mB = ppool.tile([128, WT2], F32, tag="mB")
            s_t = ppool.tile([128, WT2], F32, tag="s_t")
            tmp = ppool.tile([128, WT2], F32, tag="tmp")
            mk = ppool.tile([128, WT2], F32, tag="mk")
            t_t = ppool.tile([128, WT2], F32, tag="t_t")
            t2 = ppool.tile([128, WT2], F32, tag="t2")
            pg = ppool.tile([128, WT2], F32, tag="pg")
            scr = ppool.tile([128, WT2], F32, tag="scr")
            piS = ppool.tile([128, 16], F32, tag="piS")
            nc.sync.dma_start(piS[:, 0:15], pi_cols_d[:])
            fsb = ppool.tile([128, 30], F32, tag="fsb")
            ecol = ppool.tile([128, 1], F32, tag="ecol")
            ones_col = ppool.tile([128, 1], F32, tag="ones_col")
            nc.vector.memset(ones_col[:], 1.0)
            e_ps = pspool.tile([1, 1], F32, tag="e_ps")
            e_sb = ppool.tile([1, 1], F32, tag="e_sb")

            # group column ranges across the merged tile
            GRP = []   # (colslice, g, is_j)
            for g in range(5):
                GRP.append((slice(int(cri[g]), int(cri[g + 1])), g, False, WI[g]))
            for g in range(5):
                GRP.append((slice(WTOT + int(crj[g]), WTOT + int(crj[g + 1])),
                            g, True, WJ[g]))

            for (sl, g, isj, Wg) in GRP:
                SH = SH_J if isj else SH_I
                sh_d = posshj if isj else posshi
                for c in range(3):
                    base = (g * 3 + c) * SH
                    if isj:
                        ap = AP(sh_d, base, [[1, Wg]])
                        nc.sync.dma_start(pj[c][:, sl],
                                          ap.unsqueeze(0).partition_broadcast(128))
                    else:
                        ap = AP(sh_d, base, [[1, 128], [1, Wg]])
                        nc.sync.dma_start(pj[c][:, sl], ap)
            for (sl, g, isj, Wg) in GRP:
                for c in range(3):
                    nc.vector.tensor_scalar(
                        pj[c][:, sl], pj[c][:, sl],
                        piS[:, 3 * g + c:3 * g + c + 1], None, Alu.subtract)
            for c in range(3):
                nc.vector.tensor_scalar(mA[:], pj[c][:], 6.5, None, Alu.is_gt)
                nc.vector.tensor_scalar(mB[:], pj[c][:], -6.5, None, Alu.is_lt)
                nc.vector.scalar_tensor_tensor(
                    pj[c][:], mA[:], -13.0, pj[c][:], Alu.mult, Alu.add)
                nc.vector.scalar_tensor_tensor(
                    pj[c][:], mB[:], 13.0, pj[c][:], Alu.mult, Alu.add)
            nc.vector.tensor_tensor(s_t[:], pj[0][:], pj[0][:], Alu.mult)
            nc.vector.tensor_tensor(tmp[:], pj[1][:], pj[1][:], Alu.mult)
            nc.vector.tensor_tensor(s_t[:], s_t[:], tmp[:], Alu.add)
            nc.vector.tensor_tensor(tmp[:], pj[2][:], pj[2][:], Alu.mult)
            nc.vector.tensor_tensor(s_t[:], s_t[:], tmp[:], Alu.add)
            nc.vector.tensor_scalar(mk[:], s_t[:], T_HI, None, Alu.is_lt)
            nc.vector.scalar_tensor_tensor(
                mk[:], s_t[:], T_LO, mk[:], Alu.is_gt, Alu.mult)
            for (sl, g, isj, Wg) in GRP:
                if isj:
                    nc.vector.scalar_tensor_tensor(
                        mk[:, sl], iota_sb[:, 0:Wg],
                        thresh_sb[:, g:g + 1], mk[:, sl], Alu.is_lt, Alu.mult)
            nc.vector.tensor_scalar(s_t[:], s_t[:], S_LO, S_HI, Alu.max, Alu.min)
            nc.vector.tensor_scalar(t_t[:], s_t[:], MID, 1.0 / HALF,
                                    Alu.subtract, Alu.mult)
            nc.vector.tensor_scalar_mul(t2[:], t_t[:], 2.0)

            # Chebyshev basis recurrence (coefficient-independent, overlaps
            # the node-MLP phase):  T1 = t;  Tk = 2t*T(k-1) - T(k-2)
            Tt = [None, t_t]
            for kk in range(2, DEG_G + 1):
                tk_ = ppool.tile([128, WT2], F32, tag=f"T{kk}", name=f"T{kk}")
                nc.vector.tensor_tensor(tk_[:], t2[:], Tt[kk - 1][:], Alu.mult)
                if kk == 2:
                    nc.vector.tensor_scalar(tk_[:], tk_[:], 1.0, None,
                                            Alu.subtract)
                else:
                    nc.vector.tensor_tensor(tk_[:], tk_[:], Tt[kk - 2][:],
                                            Alu.subtract)
                Tt.append(tk_)

            # energy: B[:,k] = sum_f T_k*mask over the i-half, then one
            # dot with the broadcast u-coefficients
            Bt = ppool.tile([128, DU1], F32, tag="Bt")
            nc.vector.scalar_tensor_tensor(
                scr[:, :WTOT], mk[:, :WTOT], 1.0, mk[:, :WTOT],
                Alu.mult, Alu.mult, accum_out=Bt[:, 0:1])
            for kk in range(1, DEG_U + 1):
                nc.vector.scalar_tensor_tensor(
                    scr[:, :WTOT], Tt[kk][:, :WTOT], 1.0, mk[:, :WTOT],
                    Alu.mult, Alu.mult, accum_out=Bt[:, kk:kk + 1])
            nc.vector.scalar_tensor_tensor(
                scr[:, 0:DU1], Bt[:], 1.0, coefT[:, DP1:NCOEF],
                Alu.mult, Alu.mult, accum_out=ecol[:, 0:1])
            nc.tensor.matmul(e_ps[:], ecol[:], ones_col[:], start=True, stop=True)
            nc.scalar.copy(e_sb[:], e_ps[:])
            nc.sync.dma_start(energy_o[:], e_sb[:])

            # forces: pg = sum_k cg_k T_k, then masked per-group reductions
            nc.vector.tensor_scalar(pg[:], t_t[:], cg(1), cg(0),
                                    Alu.mult, Alu.add)
            for kk in range(2, DEG_G + 1):
                nc.vector.scalar_tensor_tensor(
                    pg[:], Tt[kk][:], cg(kk), pg[:], Alu.mult, Alu.add)
            nc.vector.tensor_tensor(pg[:], pg[:], mk[:], Alu.mult)
            for gi, (sl, g, isj, Wg) in enumerate(GRP):
                for c in range(3):
                    col = (15 if isj else 0) + 3 * g + c
                    nc.vector.scalar_tensor_tensor(
                        scr[:, sl], pg[:, sl], 1.0, pj[c][:, sl],
                        Alu.mult, Alu.mult,
                        accum_out=fsb[:, col:col + 1])
            nc.sync.dma_start(forces_i_o[:], fsb[:, 0:15])
            nc.sync.dma_start(forces_j_o[:], fsb[:, 15:30])

    _split_waits(nc)
    return nc


def kernel(positions, params, i_idx, j_idx, L, rc):
    hd = _build_host_data(positions, params, i_idx, j_idx, L, rc)

    key = ("k", hd['t_lo'], hd['t_hi'])
    if key not in _cache:
        _cache[key] = _trace_kernel(hd)
    nc = _cache[key]

    base = {
        "pi_cols": hd['pi_cols'],
        "iota": np.arange(80, dtype=np.float32),
        "ident": np.eye(128, dtype=np.float32),
        "ones_row": np.ones((1, 128), np.float32),
        "rnodes_row": hd['r_nodes'].reshape(1, 128),
        "lhsT_fit": hd['lhsT_fit'],
        "w_e1": hd['w_e1'].reshape(1, 256), "b_e1": hd['b_e1'].reshape(1, 256),
        "w_e2": hd['w_e2'], "b_e2": hd['b_e2'].reshape(1, 256),
    }
    for li in range(4):
        w, b = hd['layers_f'][li]
        base[f"w_f{li}"] = w
        base[f"b_f{li}"] = b.reshape(1, -1)
        w, b = hd['layers_e'][li]
        base[f"w_eh{li}"] = w
        base[f"b_eh{li}"] = b.reshape(1, -1)

    in_maps = []
    for c in range(N_CORES):
        m = dict(base)
        m["pos_sh_i"] = hd['pos_sh_i'][c].reshape(-1)
        m["pos_sh_j"] = hd['pos_sh_j'][c].reshape(-1)
        m["thresh_j"] = hd['thresh_j'][c]
        in_maps.append(m)

    from concourse.bass_utils import run_bass_kernel_spmd
    res = run_bass_kernel_spmd(nc, in_maps, core_ids=list(range(N_CORES)))
    global LAST_EXEC_NS
    LAST_EXEC_NS = res.exec_time_ns

    forces = np.zeros((N_ATOMS, 3), np.float64)
    energy = 0.0
    for c in range(N_CORES):
        r = res.results[c]
        fi = r["forces_i"].astype(np.float64)
        fj = r["forces_j"].astype(np.float64)
        for g in range(5):
            for comp in range(3):
                forces[128 * g:128 * (g + 1), comp] += fi[:, 3 * g + comp]
                forces[128 * g:128 * (g + 1), comp] += fj[:, 3 * g + comp]
        energy += float(r["energy"][0, 0])
    return forces.astype(np.float32), np.float32(energy)
